# revision 19
# baseline (speedup 1.0000x reference)
"""BGFusionBlock Trainium2 kernel (Bass/Tile, 8-core data parallel).

Shapes (hardcoded): aligned_feat [4, 8, 64, 128, 128] f32,
w1/w2 [64, 64, 3, 3], b1/b2 [64], wf [64, 512, 1, 1], bf [64].

Math (reference):
  emb     = conv3x3(x_t, w2) + b2           per frame
  emb_ref = conv3x3(x_t, w1) + b1           per frame
  scores[t] = <emb_ref[t], sum_j emb[j]>_c
  attn    = softmax(scores / 0.5, over t)
  out     = leaky_relu(conv1x1(x * attn, wf) + bf, 0.1)

Kernel specializations:
  - Sharding: 8 cores = (batch b, H half). Each core: T=8 frames, 64 rows.
  - emb_sum = conv3x3(sum_t x_t, w2) + 8*b2  (linearity; sum_t done on host)
  - b1 dropped (adds a t-independent constant to scores -> softmax invariant)
  - 1/temperature folded into w1
  - frames processed in pairs: SBUF layout [128p = frame f | frame f+1],
    convs via 9 block-diagonal matmuls (lhsT = diag(w_tap, w_tap))
  - matmuls in float32r (tf32); operands pre-rounded on host / by DVE ops
  - emb_ref is never materialized: each conv PSUM tile is immediately
    multiplied by emb_sum and reduced over c by a 0/1-matmul into scores
  - softmax over t in pos-major layout via DVE 32x32 stream transposes
  - attn broadcast back to [frame|frame] pair layout via 0/1-matmuls
  - weighted input x reuses the conv input chunks already in SBUF
"""

import numpy as np

B, T, C, H, W = 4, 8, 64, 128, 128
NCORES = 8
HS = 64           # rows per shard
HP, WP = HS + 2, W + 2
NT = 16           # 512-pos tiles per shard (4 rows x 128)
PAIRS = 4
TAPS = [(di, dj) for di in range(3) for dj in range(3)]

_CACHE = {}
DEBUG = False


def _round_tf32(a):
    v = np.ascontiguousarray(a, dtype=np.float32).view(np.uint32)
    half = np.uint32(1 << 12)
    lsb = (v >> np.uint32(13)) & np.uint32(1)
    v = v + half + lsb - np.uint32(1)
    v &= np.uint32(0xFFFFE000)
    return v.view(np.float32)


def _build_nc(reps: int = 1):
    from contextlib import ExitStack

    import concourse.bacc as bacc
    import concourse.bass as bass
    import concourse.tile as tile
    from concourse import mybir

    dt = mybir.dt
    AF = mybir.ActivationFunctionType
    ALU = mybir.AluOpType
    AX = mybir.AxisListType

    nc = bacc.Bacc("TRN2", target_bir_lowering=False, debug=False)
    xp_d = nc.dram_tensor("xp", [PAIRS, 128, HP, WP], dt.float16,
                          kind="ExternalInput").ap()
    xs_d = nc.dram_tensor("xs", [128, HP, WP], dt.float16,
                          kind="ExternalInput").ap()
    wp9_d = nc.dram_tensor("wp9", [128, 9, 128], dt.float16,
                           kind="ExternalInput").ap()
    ws6_d = nc.dram_tensor("ws6", [128, 6, 128], dt.float16,
                           kind="ExternalInput").ap()
    wft_d = nc.dram_tensor("wft", [128, 4, 64], dt.float16,
                           kind="ExternalInput").ap()
    ones2_d = nc.dram_tensor("ones2", [128, 4, 8], dt.float16,
                             kind="ExternalInput").ap()
    e2_d = nc.dram_tensor("e2", [8, 4, 128], dt.float16,
                          kind="ExternalInput").ap()
    b2_d = nc.dram_tensor("b2x8", [128, 1], dt.float32,
                          kind="ExternalInput").ap()
    bf_d = nc.dram_tensor("bfv", [64, 1], dt.float32,
                          kind="ExternalInput").ap()
    out_d = nc.dram_tensor("out", [64, NT * 512], dt.float32,
                           kind="ExternalOutput").ap()
    if DEBUG:
        des_d = nc.dram_tensor("dbg_es", [128, NT * 512], dt.float32,
                               kind="ExternalOutput").ap()
        dsc_d = nc.dram_tensor("dbg_sc", [32, NT * 512], dt.float32,
                               kind="ExternalOutput").ap()
        dat_d = nc.dram_tensor("dbg_at", [32, NT * 512], dt.float32,
                               kind="ExternalOutput").ap()

    def body(tc):
        with ExitStack() as ctx:
            consts = ctx.enter_context(tc.tile_pool(name="consts", bufs=1))
            persist = ctx.enter_context(tc.tile_pool(name="persist", bufs=1))

            wp9 = consts.tile([128, 9, 128], dt.float16)
            nc.sync.dma_start(wp9[:], wp9_d)
            ws6 = consts.tile([128, 6, 128], dt.float16)
            nc.sync.dma_start(ws6[:], ws6_d)
            wft = consts.tile([128, 4, 64], dt.float16)
            nc.sync.dma_start(wft[:], wft_d)
            ones2 = consts.tile([128, 4, 8], dt.float16)
            nc.sync.dma_start(ones2[:], ones2_d)
            e2 = consts.tile([8, 4, 128], dt.float16)
            nc.sync.dma_start(e2[:], e2_d)
            b2x8 = consts.tile([128, 1], dt.float32)
            nc.sync.dma_start(b2x8[:], b2_d)
            bfv = consts.tile([64, 1], dt.float32)
            nc.sync.dma_start(bfv[:], bf_d)

            emb_sum2 = persist.tile([128, NT, 512], dt.float32)

            # ---- sweep A: emb_sum = conv3x3(sum_t x, w2) + 8*b2 ----
            with ExitStack() as sctx:
                xsp = sctx.enter_context(tc.tile_pool(name="xsum", bufs=1))
                chunkp = sctx.enter_context(tc.tile_pool(name="chunk", bufs=12))
                ptp = sctx.enter_context(tc.tile_pool(name="pt", bufs=6))
                smp = sctx.enter_context(tc.tile_pool(name="sm", bufs=4))
                wtp = sctx.enter_context(tc.tile_pool(name="wt", bufs=4))
                osb = sctx.enter_context(tc.tile_pool(name="osb", bufs=3))
                psB = sctx.enter_context(
                    tc.tile_pool(name="psB", bufs=2, space="PSUM"))
                psS = sctx.enter_context(
                    tc.tile_pool(name="psS", bufs=2, space="PSUM"))
                psA = sctx.enter_context(
                    tc.tile_pool(name="psA", bufs=2, space="PSUM"))
                psO = sctx.enter_context(
                    tc.tile_pool(name="psO", bufs=2, space="PSUM"))

                xsum2 = xsp.tile([128, HP, WP], dt.float16)
                for sl0, sl1 in ((0, 17), (17, 34), (34, 51), (51, 66)):
                    nc.sync.dma_start(
                        xsum2[:, sl0:sl1, :], xs_d[:, sl0:sl1, :])

                ck = {}

                def fetch_quarter(q):
                    for k in range(PAIRS):
                        t = chunkp.tile([128, 18, WP], dt.float16, tag="ck")
                        nc.sync.dma_start(
                            t[:], xp_d[k, :, 16 * q:16 * q + 18, :])
                        ck[(q, k)] = t

                fetch_quarter(0)

                for nt in range(NT):
                    ps = psB.tile([128, 4, 128], dt.float32, tag="psB")
                    r0 = 4 * nt
                    # half2 of xsum2 is host-shifted by one element: row
                    # pairs cover taps (di,0)+(di,1) with K=128; (di,2)
                    # comes from half1 alone with K=64.
                    for di in range(3):
                        nc.tensor.matmul(
                            ps[:], ws6[:, di, :],
                            xsum2[:, r0 + di:r0 + di + 4, 0:128],
                            start=(di == 0), stop=False)
                    for di in range(3):
                        nc.tensor.matmul(
                            ps[:], ws6[0:64, 3 + di, :],
                            xsum2[0:64, r0 + di:r0 + di + 4, 2:130],
                            start=False, stop=(di == 2))
                    nc.vector.tensor_scalar_add(
                        emb_sum2[:, nt, :],
                        ps[:].rearrange("p a b -> p (a b)"), b2x8[:])
                if DEBUG:
                    nc.sync.dma_start(
                        des_d, emb_sum2[:].rearrange("p a b -> p (a b)"))

                # ---- main sweep, software-pipelined with delay 2:
                # stage S(nt): convs -> emb_ref*emb_sum -> scores -> scoresT
                # stage T(nt): softmax -> attn -> weighted -> 1x1 -> out
                scts = {}

                def stage_s(nt):
                    q, r = divmod(nt, 4)
                    if r == 0 and q + 1 <= 3:
                        fetch_quarter(q + 1)
                    pS = psS.tile([32, 512], dt.float32, tag="psS")
                    for k in range(PAIRS):
                        pB = psB.tile([128, 4, 128], dt.float32, tag="psB")
                        for j, (di, dj) in enumerate(TAPS):
                            nc.tensor.matmul(
                                pB[:], wp9[:, j, :],
                                ck[(q, k)][:, 4 * r + di:4 * r + di + 4,
                                           dj:dj + 128],
                                start=(j == 0), stop=(j == 8))
                        er = ptp.tile([128, 512], dt.float16, tag="er")
                        nc.scalar.activation(
                            er[:], pB[:].rearrange("p a b -> p (a b)"),
                            AF.Copy)
                        pt = ptp.tile([128, 512], dt.float16, tag="pt")
                        nc.gpsimd.tensor_mul(pt[:], er[:], emb_sum2[:, nt, :])
                        nc.tensor.matmul(pS[0:8, :], ones2[:, k, :],
                                         pt[:], start=(k == 0),
                                         stop=(k == 3))
                    sct = smp.tile([32, 16, 32], dt.float32, tag="scT")
                    nc.vector.transpose(
                        sct[:].rearrange("p a b -> p (a b)"), pS[:])
                    scts[nt] = sct
                    if DEBUG:
                        nc.sync.dma_start(
                            dsc_d[:, nt * 512:(nt + 1) * 512],
                            sct[:].rearrange("p a b -> p (a b)"))

                def stage_t(nt):
                    q, r = divmod(nt, 4)
                    scoresT = scts.pop(nt)
                    m = smp.tile([32, 16], dt.float32, tag="m")
                    nc.vector.tensor_reduce(
                        m[:], scoresT[:, :, 0:8], AX.X, ALU.max)
                    lmm = smp.tile([32, 16, 8], dt.float32, tag="lmm")
                    nc.vector.scalar_tensor_tensor(
                        lmm[:], scoresT[:, :, 0:8], 1.0,
                        m[:].unsqueeze(2).broadcast_to((32, 16, 8)),
                        ALU.mult, ALU.subtract)
                    eT = smp.tile([32, 16, 8], dt.float32, tag="eT")
                    nc.scalar.activation(eT[:], lmm[:], AF.Exp)
                    zT = smp.tile([32, 16], dt.float32, tag="zT")
                    nc.vector.tensor_reduce(zT[:], eT[:], AX.X, ALU.add)
                    rz = smp.tile([32, 16], dt.float32, tag="rz")
                    nc.vector.reciprocal(rz[:], zT[:])
                    attnN = smp.tile([32, 16, 32], dt.float16, tag="attnN")
                    nc.vector.tensor_tensor(
                        attnN[:, :, 0:8], eT[:],
                        rz[:].unsqueeze(2).broadcast_to((32, 16, 8)), ALU.mult)
                    atp = smp.tile([32, 512], dt.float16, tag="atp")
                    nc.vector.transpose(
                        atp[:], attnN[:].rearrange("p a b -> p (a b)"))

                    pO = psO.tile([64, 4, 128], dt.float32, tag="psO")
                    for k in range(PAIRS):
                        pA = psA.tile([128, 4, 128], dt.float32, tag="psA")
                        nc.tensor.matmul(pA[:], e2[:, k, :], atp[0:8, :],
                                         start=True, stop=True)
                        wt = wtp.tile([128, 4, 128], dt.float16, tag="wt")
                        nc.vector.tensor_mul(
                            wt[:], ck[(q, k)][:, 4 * r + 1:4 * r + 5, 1:129],
                            pA[:])
                        nc.tensor.matmul(pO[:], wft[:, k, :], wt[:],
                                         start=(k == 0), stop=(k == 3))
                    vt = osb.tile([64, 4, 128], dt.float32, tag="vt")
                    nc.scalar.add(vt[:], pO[:], bfv[:])
                    ot = osb.tile([64, 4, 128], dt.float32, tag="ot")
                    # leaky_relu(v, 0.1) == max(0.1*v, v)
                    nc.vector.scalar_tensor_tensor(
                        ot[:], vt[:], 0.1, vt[:], ALU.mult, ALU.max)
                    nc.sync.dma_start(
                        out_d[:, nt * 512:(nt + 1) * 512],
                        ot[:].rearrange("p a b -> p (a b)"))

                DELAY = 2
                for nt in range(NT + DELAY):
                    if nt < NT:
                        stage_s(nt)
                    if nt >= DELAY:
                        stage_t(nt - DELAY)

    with tile.TileContext(nc) as tc:
        if reps == 1:
            body(tc)
        else:
            with tc.For_i(0, reps, 1,
                          hint_engines=(mybir.EngineType.PE,
                                        mybir.EngineType.DVE,
                                        mybir.EngineType.Activation,
                                        mybir.EngineType.Pool,
                                        mybir.EngineType.SP)):
                body(tc)
    nc.compile()
    return nc


def _prep_inputs(aligned_feat, w1, b1, w2, b2, wf, bf):
    x = np.ascontiguousarray(aligned_feat, dtype=np.float32)
    w1 = np.asarray(w1, np.float32)
    w2 = np.asarray(w2, np.float32)
    b2 = np.asarray(b2, np.float32)
    wf = np.asarray(wf, np.float32)
    bf = np.asarray(bf, np.float32)

    xpad = np.zeros((B, T, C, H + 2, W + 2), np.float32)
    xpad[:, :, :, 1:H + 1, 1:W + 1] = x
    xspad = np.zeros((B, C, H + 2, W + 2), np.float32)
    xspad[:, :, 1:H + 1, 1:W + 1] = x.sum(axis=1)

    w1s = (2.0 * w1)  # fold 1/temperature
    wp9 = np.zeros((128, 9, 128), np.float32)
    ws6 = np.zeros((128, 6, 128), np.float32)
    for j, (di, dj) in enumerate(TAPS):
        wp9[0:64, j, 0:64] = w1s[:, :, di, dj].T
        wp9[64:128, j, 64:128] = w1s[:, :, di, dj].T
    for di in range(3):
        # K=128 pair: rows 0-63 tap (di,0), rows 64-127 tap (di,1);
        # output columns duplicated [emb_sum | emb_sum]
        ws6[0:64, di, 0:64] = w2[:, :, di, 0].T
        ws6[0:64, di, 64:128] = w2[:, :, di, 0].T
        ws6[64:128, di, 0:64] = w2[:, :, di, 1].T
        ws6[64:128, di, 64:128] = w2[:, :, di, 1].T
        # K=64 single: tap (di,2) from half1
        ws6[0:64, 3 + di, 0:64] = w2[:, :, di, 2].T
        ws6[0:64, 3 + di, 64:128] = w2[:, :, di, 2].T
    wfm = wf.reshape(C, T, C)  # [o, t, c]
    wft = np.zeros((128, 4, 64), np.float32)
    for k in range(PAIRS):
        wft[0:64, k, :] = wfm[:, 2 * k, :].T
        wft[64:128, k, :] = wfm[:, 2 * k + 1, :].T
    ones2 = np.zeros((128, 4, 8), np.float32)
    for k in range(PAIRS):
        ones2[0:64, k, 2 * k] = 1.0
        ones2[64:128, k, 2 * k + 1] = 1.0
    e2 = np.zeros((8, 4, 128), np.float32)
    for k in range(PAIRS):
        e2[2 * k, k, 0:64] = 1.0
        e2[2 * k + 1, k, 64:128] = 1.0
    b2x8 = np.tile(8.0 * b2, 2).reshape(128, 1)
    bfv = bf.reshape(64, 1)

    shared = {
        "wp9": wp9.astype(np.float16), "ws6": ws6.astype(np.float16),
        "wft": wft.astype(np.float16), "ones2": ones2.astype(np.float16),
        "e2": e2.astype(np.float16),
        "b2x8": np.ascontiguousarray(b2x8), "bfv": np.ascontiguousarray(bfv),
    }
    in_maps = []
    for core in range(NCORES):
        b, s = divmod(core, 2)
        r0 = HS * s
        xp = np.empty((PAIRS, 128, HP, WP), np.float16)
        for k in range(PAIRS):
            xp[k, 0:64] = xpad[b, 2 * k, :, r0:r0 + HP, :]
            xp[k, 64:128] = xpad[b, 2 * k + 1, :, r0:r0 + HP, :]
        xs = np.empty((128, HP, WP), np.float16)
        xs[0:64] = xspad[b, :, r0:r0 + HP, :]
        # half2 = half1 shifted by one element (for the tap-pair trick)
        flat1 = xs[0:64].reshape(64, HP * WP)
        flat2 = xs[64:128].reshape(64, HP * WP)
        flat2[:, :-1] = flat1[:, 1:]
        flat2[:, -1] = 0
        in_maps.append({"xp": xp, "xs": xs, **shared})
    return in_maps


def _get_nc(reps: int = 1):
    if DEBUG:
        return _build_nc(reps)
    if reps not in _CACHE:
        _CACHE[reps] = _build_nc(reps)
    return _CACHE[reps]


def _run(in_maps, reps: int = 1):
    from concourse import bass_utils
    nc = _get_nc(reps)
    return bass_utils.run_bass_kernel_spmd(
        nc, in_maps, core_ids=list(range(NCORES)), trace=False)


def kernel(aligned_feat, w1, b1, w2, b2, wf, bf):
    in_maps = _prep_inputs(aligned_feat, w1, b1, w2, b2, wf, bf)
    res = _run(in_maps)
    out = np.empty((B, C, H, W), np.float32)
    for core in range(NCORES):
        b, s = divmod(core, 2)
        out[b, :, HS * s:HS * (s + 1), :] = \
            res.results[core]["out"].reshape(C, HS, W)
    return out
